# revision 23
# baseline (speedup 1.0000x reference)
"""AcausalCrosscoder (topk) Trainium2 kernel — 8-core data-parallel over batch.

Math (per batch row):
    pre  = X @ W_enc + b_enc          (X: [B, D=MLD=8192], W_enc: [D, H=16384])
    h    = topk_mask(pre, k=64)       (keep top-64 per row, zero elsewhere)
    out  = h @ W_dec + b_dec          (W_dec: [H, D])

Precision scheme (fp32-exact top-k selection):
    X = Xr + dX, W = Wr + dW with Xr/Wr rounded to fp32r (1s8e11m).
    pre = Xr@Wr (fp32r matmul, exact products, fp32 PSUM accumulation)
        + fp8(dX*2^13)@fp8(W*2^9) + fp8(X)@fp8(dW*2^22)   (e4m3 DoubleRow
          corrections at 2x matmul rate, both at a common 2^22 product scale)
    The whole encode runs at a 2^22 scale (Xr*2^11 @ Wr*2^11) so main and
    corrections share one PSUM accumulation group; the scale is folded into
    W_dec (*2^-22) on the host so decode output is unscaled.
    Residual selection noise ~1.5e-6 vs a ~1.3e-3 median top-64 boundary gap.

Top-k: during encode evacuation each 512-wide pre chunk gets a top-16
candidate extraction (max8 + match_replace + max8, hidden under the PE).
The candidate array is compacted incrementally (running top-64 after
h-groups 7 and 13, on DVE time hidden under the encode matmuls) so the
post-encode merge only scans 128 surviving values before yielding the
64th-largest value t per row; hidden = (pre >= t) * pre is applied with one
fused scalar_tensor_tensor op per tile (bf16 out).  The hidden tiles are
transposed into the decode layout with xbar DMA transposes (SBUF->SBUF, off
the PE critical path), so the tensor engine goes straight from the encode
matmuls to the decode matmuls; the first decode weight tiles are prefetched
during the threshold phase.  Decode streams W_dec once with the transposed
hidden resident in SBUF (per-(m, e) [128, 16, 128] tiles).

Each core handles B/8 = 512 rows; weights are replicated. No collectives.
"""

import numpy as np

import ml_dtypes

import concourse.bass as bass
import concourse.mybir as mybir
import concourse.tile as tile
from concourse import bacc
from concourse.bass import ds
from concourse.bass_utils import run_bass_kernel_spmd

N_CORES = 8
_B, _M, _L, _DM = 4096, 2, 4, 1024
_D = _M * _L * _DM  # 8192
_H = 16384
_TOPK = 64


def round_fp32r(x: np.ndarray) -> np.ndarray:
    """Round fp32 -> fp32r (1s, 8e, 11m stored in top 20 bits), RNE."""
    x = np.ascontiguousarray(x, np.float32)
    bits = x.view(np.uint32).astype(np.uint64)
    lsb = (bits >> np.uint64(12)) & np.uint64(1)
    rounded = (bits + np.uint64(0x7FF) + lsb) & np.uint64(0xFFFFF000)
    return rounded.astype(np.uint32).view(np.float32).reshape(x.shape)


def _bf16(x: np.ndarray) -> np.ndarray:
    return np.asarray(x, np.float32).astype(ml_dtypes.bfloat16)


def _e4(x: np.ndarray) -> np.ndarray:
    return np.asarray(x, np.float32).astype(ml_dtypes.float8_e4m3)


def build_nc(BC=_B // N_CORES, D=_D, H=_H, topk=_TOPK, decode_dt="bfloat16"):
    """Build the per-core Bass program (SPMD; all cores run the same NEFF)."""
    f32 = mybir.dt.float32
    f32r = mybir.dt.float32r
    e4 = mybir.dt.float8e4
    ddt = getattr(mybir.dt, decode_dt)
    DR = mybir.MatmulPerfMode.DoubleRow
    is_ge = mybir.AluOpType.is_ge
    mult = mybir.AluOpType.mult

    KT = D // 128    # 64 encode contraction chunks
    KT2 = KT // 2    # 32 fp8 pair chunks
    KH = H // 128    # 128 decode contraction chunks
    MB = BC // 128   # 4 batch blocks per core
    HG = H // 1024   # 16 encode h-groups (2 x 512 tiles each)
    NCH = H // 512   # 32 candidate chunks per row
    assert topk == 64 and MB == 4

    nc = bacc.Bacc("TRN2", target_bir_lowering=False)

    xtr_p = nc.declare_dram_parameter("xtr", [D, BC], f32r, isOutput=False)
    x8_p = nc.declare_dram_parameter("x8", [D, BC], e4, isOutput=False)
    dx8_p = nc.declare_dram_parameter("dx8", [D, BC], e4, isOutput=False)
    wr_p = nc.declare_dram_parameter("wr", [D, H], f32r, isOutput=False)
    w8_p = nc.declare_dram_parameter("w8", [D, H], e4, isOutput=False)
    dw8_p = nc.declare_dram_parameter("dw8", [D, H], e4, isOutput=False)
    wd_p = nc.declare_dram_parameter("wd", [H, D], ddt, isOutput=False)
    benc_p = nc.declare_dram_parameter("benc", [H], f32, isOutput=False)
    bdec_p = nc.declare_dram_parameter("bdec", [D], f32, isOutput=False)
    out_p = nc.declare_dram_parameter("out", [BC, D], f32, isOutput=True)

    pre_dram = nc.dram_tensor("pre_scratch", [MB, 128, H], f32)

    with tile.TileContext(nc) as tc:
        with tc.tile_pool(name="persist", bufs=1) as pers:
            cands = [
                pers.tile([128, NCH, 16], f32, name=f"cand{m}") for m in range(MB)
            ]
            m8s = [pers.tile([128, 8], f32, name=f"m8_{m}") for m in range(MB)]
            cmps = [pers.tile([128, 4, 16], f32, name=f"cmp_{m}") for m in range(MB)]

            def _compact_cands(m, lo, hi):
                """Extract the top-64 of cands[m][:, lo:hi] into the last 4
                chunk slots of that range, zeroing the rest, so later merges
                only scan the surviving candidates.  Runs on DVE, hidden
                under the encode matmuls of the following h-groups."""
                v = cands[m][:, ds(lo, hi - lo), :]
                for r in range(8):
                    nc.vector.max(cmps[m][:, r % 4, ds((r // 4) * 8, 8)], v)
                    if r < 7:
                        nc.vector.match_replace(
                            v, cmps[m][:, r % 4, ds((r // 4) * 8, 8)], v, 0.0
                        )
                nc.vector.tensor_scalar_mul(v, v, 0.0)
                nc.vector.tensor_scalar_add(
                    cands[m][:, ds(hi - 4, 4), :], cmps[m][:], 0.0
                )

            # ---------------- phase 1: encode ----------------
            with (
                tc.tile_pool(name="xres", bufs=1) as xres,
                tc.tile_pool(name="wstream", bufs=6) as wpool,
                tc.tile_pool(name="w8stream", bufs=5) as w8pool,
                tc.tile_pool(name="xstream", bufs=6) as xpool,
                tc.tile_pool(name="epsum", bufs=8, space="PSUM") as pse,
                tc.tile_pool(name="evac", bufs=10) as evac,
                tc.tile_pool(name="cscr", bufs=2) as cscr,
                tc.tile_pool(name="ebias", bufs=2) as ebias,
            ):
                # resident fp8 X tiles, loaded in per-kt2 chunks during hg 0
                # so the small chunk DMAs never sit in front of the weight /
                # xtr streams in the queue (each chunk lands well within the
                # fp32r matmul time of its own kt2 step)
                x8 = xres.tile([128, KT, BC], e4, name="x8_res")
                dx8 = xres.tile([128, KT, BC], e4, name="dx8_res")

                def _load_x8_chunk(c):
                    nc.gpsimd.dma_start(
                        dx8[:, ds(c * 2, 2), :],
                        dx8_p[ds(c * 256, 256), :].rearrange(
                            "(kt p) b -> p kt b", p=128
                        ),
                    )
                    nc.scalar.dma_start(
                        x8[:, ds(c * 2, 2), :],
                        x8_p[ds(c * 256, 256), :].rearrange(
                            "(kt p) b -> p kt b", p=128
                        ),
                    )

                for hg in range(HG):
                    pss = [
                        [
                            pse.tile([128, 512], f32, name=f"eps_{m}_{j}", tag="eps")
                            for j in range(2)
                        ]
                        for m in range(MB)
                    ]
                    benc_t = ebias.tile([128, 2, 512], f32, tag="benc", name="benc_t")
                    for kt2 in range(KT2):
                        if kt2 == 1:
                            # deferred so kt2=0's weight tiles lead the ring;
                            # benc is only consumed at end-of-hg evacuation
                            nc.gpsimd.dma_start(
                                benc_t[:],
                                benc_p[ds(hg * 1024, 1024)]
                                .rearrange("(j n) -> j n", n=512)
                                .partition_broadcast(128),
                            )
                        wr_t = wpool.tile([128, 2, 1024], f32r, tag="wr", name="wr_t")
                        nc.sync.dma_start(
                            wr_t[:],
                            wr_p[ds(kt2 * 256, 256), ds(hg * 1024, 1024)].rearrange(
                                "(k p) n -> p k n", p=128
                            ),
                        )
                        w8_t = w8pool.tile([128, 2, 1024], e4, tag="w8", name="w8_t")
                        nc.gpsimd.dma_start(
                            w8_t[:],
                            w8_p[ds(kt2 * 256, 256), ds(hg * 1024, 1024)].rearrange(
                                "(k p) n -> p k n", p=128
                            ),
                        )
                        dw8_t = w8pool.tile([128, 2, 1024], e4, tag="dw8", name="dw8_t")
                        nc.gpsimd.dma_start(
                            dw8_t[:],
                            dw8_p[ds(kt2 * 256, 256), ds(hg * 1024, 1024)].rearrange(
                                "(k p) n -> p k n", p=128
                            ),
                        )
                        xtr_t = xpool.tile([128, 2, BC], f32r, tag="xtr", name="xtr_t")
                        nc.scalar.dma_start(
                            xtr_t[:],
                            xtr_p[ds(kt2 * 256, 256), :].rearrange(
                                "(k p) b -> p k b", p=128
                            ),
                        )
                        if hg == 0:
                            if kt2 == 0:
                                _load_x8_chunk(0)
                            if kt2 + 1 < KT2:
                                _load_x8_chunk(kt2 + 1)

                        first = kt2 == 0
                        last = kt2 == KT2 - 1
                        for kk in range(2):
                            for m in range(MB):
                                for j in range(2):
                                    nc.tensor.matmul(
                                        pss[m][j][:],
                                        xtr_t[:, kk, ds(m * 128, 128)],
                                        wr_t[:, kk, ds(j * 512, 512)],
                                        start=(first and kk == 0),
                                        stop=False,
                                    )
                        for m in range(MB):
                            for j in range(2):
                                nc.tensor.matmul(
                                    pss[m][j][:],
                                    dx8[:, ds(kt2 * 2, 2), ds(m * 128, 128)],
                                    w8_t[:, :, ds(j * 512, 512)],
                                    start=False,
                                    stop=False,
                                    perf_mode=DR,
                                )
                        for m in range(MB):
                            for j in range(2):
                                nc.tensor.matmul(
                                    pss[m][j][:],
                                    x8[:, ds(kt2 * 2, 2), ds(m * 128, 128)],
                                    dw8_t[:, :, ds(j * 512, 512)],
                                    start=False,
                                    stop=last,
                                    perf_mode=DR,
                                )
                    # free all 8 PSUM banks first (bias adds), then do the
                    # candidate extraction on the SBUF staging tiles so the
                    # next h-group's matmuls aren't gated on DVE extract work
                    sts = {}
                    for m in range(MB):
                        for j in range(2):
                            st = evac.tile([128, 512], f32, tag="est", name="est")
                            if (m * 2 + j) % 2 == 0:
                                nc.vector.tensor_add(st[:], pss[m][j][:], benc_t[:, j])
                            else:
                                nc.scalar.activation(
                                    st[:],
                                    pss[m][j][:],
                                    mybir.ActivationFunctionType.Identity,
                                    bias=0.0,
                                    scale=1.0,
                                )
                                nc.vector.tensor_add(st[:], st[:], benc_t[:, j])
                            nc.gpsimd.dma_start(
                                pre_dram[m, :, ds(hg * 1024 + j * 512, 512)], st[:]
                            )
                            sts[(m, j)] = st
                    for m in range(MB):
                        for j in range(2):
                            # top-16 candidate extraction for this 512 chunk
                            st = sts[(m, j)]
                            c = hg * 2 + j
                            cs = cscr.tile([128, 512], f32, tag="cs", name="cs")
                            nc.vector.max(cands[m][:, c, ds(0, 8)], st[:])
                            nc.vector.match_replace(
                                cs[:], cands[m][:, c, ds(0, 8)], st[:], 0.0
                            )
                            nc.vector.max(cands[m][:, c, ds(8, 8)], cs[:])
                    # incremental candidate compaction: keep the running
                    # top-64 so the final post-encode merge scans only 128
                    # values (chunks 24..31) instead of all 512
                    if hg == 7:
                        for m in range(MB):
                            _compact_cands(m, 0, 16)
                    elif hg == 13:
                        for m in range(MB):
                            _compact_cands(m, 12, 28)

            # ----- phases 2+3: threshold -> hidden^T (xbar DMA) -> decode -----
            # one container so the decode weight stream can prefetch during
            # the threshold phase; hidden^T lives in four [128, KH, 128]
            # per-m tiles whose per-(e, m) transpose destinations are
            # contiguous (required for correct xbar DMA transposes)
            with tc.tile_pool(name="htres", bufs=1) as htres:
                # per-(m, e) tiles so each xbar transpose writes one whole
                # tile — decode's kh-chunk deps are then trivially precise
                # and its first matmuls only wait for the e=0 transposes
                NE = H // 2048  # 8
                hts = [
                    [
                        htres.tile([128, 16, 128], ddt, name=f"ht_{m}_{e}")
                        for e in range(NE)
                    ]
                    for m in range(MB)
                ]

                with (
                    tc.tile_pool(name="prel", bufs=2) as prel,
                    tc.tile_pool(name="hidp", bufs=3) as hidp,
                    tc.tile_pool(name="wdp", bufs=8) as wdp,
                    tc.tile_pool(name="dpsum", bufs=8, space="PSUM") as dps,
                    tc.tile_pool(name="devac", bufs=6) as devac,
                    tc.tile_pool(name="dbias", bufs=2) as dbias,
                ):
                    E8 = 2048
                    NG = D // 1024  # 8
                    # 8-round merge over the surviving candidates (compacted
                    # top-64 in chunks 24..27 + fresh chunks 28..31) -> 64th
                    for m in range(MB):
                        for r in range(8):
                            nc.vector.max(m8s[m][:], cands[m][:, ds(24, 8), :])
                            if r < 7:
                                nc.vector.match_replace(
                                    cands[m][:, ds(24, 8), :],
                                    m8s[m][:],
                                    cands[m][:, ds(24, 8), :],
                                    0.0,
                                )

                    # prefetch the first decode weight tiles + bias so the
                    # tensor engine can roll straight into decode
                    PREF = 8
                    bdec0 = dbias.tile([128, 2, 512], f32, tag="bdec", name="bdec_t")
                    nc.scalar.dma_start(
                        bdec0[:],
                        bdec_p[ds(0, 1024)]
                        .rearrange("(j n) -> j n", n=512)
                        .partition_broadcast(128),
                    )
                    wd_pref = []
                    for kh in range(PREF):
                        wd_t = wdp.tile([128, 1024], ddt, tag="wd", name="wd_t")
                        nc.sync.dma_start(
                            wd_t[:], wd_p[ds(kh * 128, 128), ds(0, 1024)]
                        )
                        wd_pref.append(wd_t)

                    # threshold-apply, then xbar-transpose each [128, 2048]
                    # hidden tile into the decode layout (no PE involvement)
                    for e in range(NE):
                        for m in range(MB):
                            t_ap = m8s[m][:, ds(7, 1)]
                            pt = prel.tile([128, E8], f32, tag="pt", name="pt")
                            # split the pre readback across two rings so ht
                            # production outpaces decode consumption; the
                            # transposes (scalar) never head-of-line block it
                            pt_eng = nc.gpsimd if m < 2 else nc.scalar
                            pt_eng.dma_start(
                                pt[:], pre_dram[m, :, ds(e * E8, E8)]
                            )
                            hd = hidp.tile([128, E8], ddt, tag="hd", name="hd")
                            nc.vector.scalar_tensor_tensor(
                                hd[:], pt[:], t_ap, pt[:], is_ge, mult
                            )
                            nc.scalar.dma_start_transpose(
                                hts[m][e][:], hd[:]
                            )

                    # ---------------- phase 3: decode ----------------
                    for ng in range(NG):
                        pss = [
                            [
                                dps.tile(
                                    [128, 512], f32, name=f"dps_{m}_{j}", tag="dps"
                                )
                                for j in range(2)
                            ]
                            for m in range(MB)
                        ]
                        if ng == 0:
                            bdec_t = bdec0
                        else:
                            bdec_t = dbias.tile(
                                [128, 2, 512], f32, tag="bdec", name="bdec_t"
                            )
                            nc.scalar.dma_start(
                                bdec_t[:],
                                bdec_p[ds(ng * 1024, 1024)]
                                .rearrange("(j n) -> j n", n=512)
                                .partition_broadcast(128),
                            )
                        for kh in range(KH):
                            if ng == 0 and kh < PREF:
                                wd_t = wd_pref[kh]
                            else:
                                wd_t = wdp.tile(
                                    [128, 1024], ddt, tag="wd", name="wd_t"
                                )
                                nc.sync.dma_start(
                                    wd_t[:],
                                    wd_p[ds(kh * 128, 128), ds(ng * 1024, 1024)],
                                )
                            first = kh == 0
                            last = kh == KH - 1
                            for m in range(MB):
                                for j in range(2):
                                    nc.tensor.matmul(
                                        pss[m][j][:],
                                        hts[m][kh // 16][:, kh % 16, :],
                                        wd_t[:, ds(j * 512, 512)],
                                        start=first,
                                        stop=last,
                                    )
                        for m in range(MB):
                            for j in range(2):
                                st = devac.tile([128, 512], f32, tag="dst", name="dst")
                                # split psum-freeing adds across DVE and ACT so
                                # the next ng group's matmuls unblock sooner
                                if (m * 2 + j) % 2 == 0:
                                    nc.vector.tensor_add(
                                        st[:], pss[m][j][:], bdec_t[:, j]
                                    )
                                else:
                                    nc.scalar.activation(
                                        st[:],
                                        pss[m][j][:],
                                        mybir.ActivationFunctionType.Identity,
                                        bias=0.0,
                                        scale=1.0,
                                    )
                                    nc.vector.tensor_add(st[:], st[:], bdec_t[:, j])
                                nc.scalar.dma_start(
                                    out_p[
                                        ds(m * 128, 128), ds(ng * 1024 + j * 512, 512)
                                    ],
                                    st[:],
                                )

    nc.compile()
    return nc


def prepare_inputs(X, W_enc, W_dec, b_enc, b_dec, n_cores=N_CORES):
    """Host-side dtype splits + per-core sharding. X: [B, D]."""
    B, D = X.shape
    BC = B // n_cores

    Wr = round_fp32r(W_enc)
    wr = (Wr * np.float32(2048.0)).astype(np.float32)  # fp32r * 2^11
    w8 = _e4(W_enc * np.float32(2.0**9))
    dw8 = _e4((W_enc - Wr) * np.float32(2.0**22))
    wd = _bf16(np.asarray(W_dec, np.float32) * np.float32(2.0**-22))
    benc = (np.ascontiguousarray(b_enc, np.float32) * np.float32(2.0**22)).astype(
        np.float32
    )
    bdec = np.ascontiguousarray(b_dec, np.float32).reshape(D)

    in_maps = []
    for c in range(n_cores):
        XT = np.ascontiguousarray(X[c * BC : (c + 1) * BC].T)  # [D, BC]
        XTr = round_fp32r(XT)
        in_maps.append(
            {
                "xtr": (XTr * np.float32(2048.0)).astype(np.float32),
                "x8": _e4(XT),
                "dx8": _e4((XT - XTr) * np.float32(2.0**13)),
                "wr": wr,
                "w8": w8,
                "dw8": dw8,
                "wd": wd,
                "benc": benc,
                "bdec": bdec,
            }
        )
    return in_maps


_NC_CACHE = {}


def _get_nc(**kw):
    key = tuple(sorted(kw.items()))
    if key not in _NC_CACHE:
        _NC_CACHE[key] = build_nc(**kw)
    return _NC_CACHE[key]


def kernel(activation_BMLD, W_enc_MLDH, W_dec_HMLD, b_enc_H, b_dec_MLD, k, **run_kw):
    assert int(k) == _TOPK
    B = activation_BMLD.shape[0]
    X = np.ascontiguousarray(activation_BMLD, np.float32).reshape(B, _D)
    W_enc = np.ascontiguousarray(W_enc_MLDH, np.float32).reshape(_D, _H)
    W_dec = np.ascontiguousarray(W_dec_HMLD, np.float32).reshape(_H, _D)

    nc = _get_nc(BC=B // N_CORES)
    in_maps = prepare_inputs(X, W_enc, W_dec, b_enc_H, b_dec_MLD)
    res = run_bass_kernel_spmd(nc, in_maps, core_ids=list(range(N_CORES)), **run_kw)
    out = np.concatenate([res.results[c]["out"] for c in range(N_CORES)], axis=0)
    if run_kw.get("trace"):
        kernel.last_result = res
    return out.reshape(B, _M, _L, _DM).astype(np.float32)


# revision 35
# speedup vs baseline: 1.0395x; 1.0395x over previous
"""AcausalCrosscoder (topk) Trainium2 kernel — 8-core data-parallel over batch.

Math (per batch row):
    pre  = X @ W_enc + b_enc          (X: [B, D=MLD=8192], W_enc: [D, H=16384])
    h    = topk_mask(pre, k=64)       (keep top-64 per row, zero elsewhere)
    out  = h @ W_dec + b_dec          (W_dec: [H, D])

Precision scheme (fp32-exact top-k selection):
    X = Xr + dX, W = Wr + dW with Xr/Wr rounded to fp32r (1s8e11m).
    pre = Xr@Wr (fp32r matmul, exact products, fp32 PSUM accumulation)
        + fp8(dX*2^13)@fp8(W*2^9) + fp8(X)@fp8(dW*2^22)   (e4m3 DoubleRow
          corrections at 2x matmul rate, both at a common 2^22 product scale)
    The whole encode runs at a 2^22 scale (Xr*2^11 @ Wr*2^11) so main and
    corrections share one PSUM accumulation group; the scale is folded into
    W_dec (*2^-22) on the host so decode output is unscaled.
    Residual selection noise ~1.5e-6 vs a ~1.3e-3 median top-64 boundary gap.

Top-k: during encode evacuation each 512-wide pre chunk gets a top-16
candidate extraction (max8 + match_replace + max8, hidden under the PE).
The candidate array is compacted incrementally (running top-64 after
h-groups 7 and 13, on DVE time hidden under the encode matmuls) so the
post-encode merge only scans 128 surviving values before yielding the
64th-largest value t per row; hidden = (pre >= t) * pre is applied with one
fused scalar_tensor_tensor op per tile (bf16 out).  The hidden tiles are
transposed into the decode layout with xbar DMA transposes (SBUF->SBUF, off
the PE critical path), so the tensor engine goes straight from the encode
matmuls to the decode matmuls; the first decode weight tiles are prefetched
during the threshold phase.  Decode streams W_dec once with the transposed
hidden resident in SBUF (per-(m, e) [128, 16, 128] tiles).

Each core handles B/8 = 512 rows; weights are replicated. No collectives.
"""

import numpy as np

import ml_dtypes

import concourse.bass as bass
import concourse.mybir as mybir
import concourse.tile as tile
from concourse import bacc
from concourse.bass import ds
from concourse.bass_utils import run_bass_kernel_spmd

N_CORES = 8
_B, _M, _L, _DM = 4096, 2, 4, 1024
_D = _M * _L * _DM  # 8192
_H = 16384
_TOPK = 64


def round_fp32r(x: np.ndarray) -> np.ndarray:
    """Round fp32 -> fp32r (1s, 8e, 11m stored in top 20 bits), RNE."""
    x = np.ascontiguousarray(x, np.float32)
    bits = x.view(np.uint32).astype(np.uint64)
    lsb = (bits >> np.uint64(12)) & np.uint64(1)
    rounded = (bits + np.uint64(0x7FF) + lsb) & np.uint64(0xFFFFF000)
    return rounded.astype(np.uint32).view(np.float32).reshape(x.shape)


def _bf16(x: np.ndarray) -> np.ndarray:
    return np.asarray(x, np.float32).astype(ml_dtypes.bfloat16)


def _e4(x: np.ndarray) -> np.ndarray:
    return np.asarray(x, np.float32).astype(ml_dtypes.float8_e4m3)


def build_nc(BC=_B // N_CORES, D=_D, H=_H, topk=_TOPK, decode_dt="bfloat16"):
    """Build the per-core Bass program (SPMD; all cores run the same NEFF)."""
    f32 = mybir.dt.float32
    f32r = mybir.dt.float32r
    e4 = mybir.dt.float8e4
    ddt = getattr(mybir.dt, decode_dt)
    DR = mybir.MatmulPerfMode.DoubleRow
    is_ge = mybir.AluOpType.is_ge
    mult = mybir.AluOpType.mult

    KT = D // 128    # 64 encode contraction chunks
    KT2 = KT // 2    # 32 fp8 pair chunks
    KH = H // 128    # 128 decode contraction chunks
    MB = BC // 128   # 4 batch blocks per core
    HG = H // 1024   # 16 encode h-groups (2 x 512 tiles each)
    NCH = H // 512   # 32 candidate chunks per row
    assert topk == 64 and MB == 4

    nc = bacc.Bacc("TRN2", target_bir_lowering=False)

    xtr_p = nc.declare_dram_parameter("xtr", [D, BC], f32r, isOutput=False)
    x8_p = nc.declare_dram_parameter("x8", [D, BC], e4, isOutput=False)
    dx8_p = nc.declare_dram_parameter("dx8", [D, BC], e4, isOutput=False)
    wr_p = nc.declare_dram_parameter("wr", [D, H], f32r, isOutput=False)
    w8_p = nc.declare_dram_parameter("w8", [D, H], e4, isOutput=False)
    dw8_p = nc.declare_dram_parameter("dw8", [D, H], e4, isOutput=False)
    wd_p = nc.declare_dram_parameter("wd", [H, D], ddt, isOutput=False)
    benc_p = nc.declare_dram_parameter("benc", [H], f32, isOutput=False)
    bdec_p = nc.declare_dram_parameter("bdec", [D], f32, isOutput=False)
    out_p = nc.declare_dram_parameter("out", [BC, D], f32, isOutput=True)

    pre_dram = nc.dram_tensor("pre_scratch", [MB, 128, H], f32)

    with tile.TileContext(nc) as tc:
        with tc.tile_pool(name="persist", bufs=1) as pers:
            cands = [
                pers.tile([128, NCH, 16], f32, name=f"cand{m}") for m in range(MB)
            ]
            m8s = [pers.tile([128, 8], f32, name=f"m8_{m}") for m in range(MB)]
            cmps = [pers.tile([128, 4, 16], f32, name=f"cmp_{m}") for m in range(MB)]

            def _compact_cands(m, lo, hi):
                """Extract the top-64 of cands[m][:, lo:hi] into the last 4
                chunk slots of that range, zeroing the rest, so later merges
                only scan the surviving candidates.  Runs on DVE, hidden
                under the encode matmuls of the following h-groups."""
                v = cands[m][:, ds(lo, hi - lo), :]
                for r in range(8):
                    nc.vector.max(cmps[m][:, r % 4, ds((r // 4) * 8, 8)], v)
                    if r < 7:
                        nc.vector.match_replace(
                            v, cmps[m][:, r % 4, ds((r // 4) * 8, 8)], v, 0.0
                        )
                nc.vector.tensor_scalar_mul(v, v, 0.0)
                nc.vector.tensor_scalar_add(
                    cands[m][:, ds(hi - 4, 4), :], cmps[m][:], 0.0
                )

            # ---------------- phase 1: encode ----------------
            with (
                tc.tile_pool(name="xres", bufs=1) as xres,
                tc.tile_pool(name="wstream", bufs=6) as wpool,
                tc.tile_pool(name="w8stream", bufs=5) as w8pool,
                tc.tile_pool(name="xstream", bufs=6) as xpool,
                tc.tile_pool(name="epsum", bufs=8, space="PSUM") as pse,
                tc.tile_pool(name="evac", bufs=10) as evac,
                tc.tile_pool(name="cscr", bufs=2) as cscr,
                tc.tile_pool(name="ebias", bufs=2) as ebias,
            ):
                # resident fp8 X tiles, loaded in per-kt2 chunks during hg 0
                # so the small chunk DMAs never sit in front of the weight /
                # xtr streams in the queue (each chunk lands well within the
                # fp32r matmul time of its own kt2 step)
                x8 = xres.tile([128, KT, BC], e4, name="x8_res")
                dx8 = xres.tile([128, KT, BC], e4, name="dx8_res")

                def _load_x8_chunk(c):
                    nc.gpsimd.dma_start(
                        dx8[:, ds(c * 2, 2), :],
                        dx8_p[ds(c * 256, 256), :].rearrange(
                            "(kt p) b -> p kt b", p=128
                        ),
                    )
                    nc.scalar.dma_start(
                        x8[:, ds(c * 2, 2), :],
                        x8_p[ds(c * 256, 256), :].rearrange(
                            "(kt p) b -> p kt b", p=128
                        ),
                    )

                for hg in range(HG):
                    pss = [
                        [
                            pse.tile([128, 512], f32, name=f"eps_{m}_{j}", tag="eps")
                            for j in range(2)
                        ]
                        for m in range(MB)
                    ]
                    benc_t = ebias.tile([128, 2, 512], f32, tag="benc", name="benc_t")
                    nc.gpsimd.dma_start(
                        benc_t[:],
                        benc_p[ds(hg * 1024, 1024)]
                        .rearrange("(j n) -> j n", n=512)
                        .partition_broadcast(128),
                    )
                    for kt2 in range(KT2):
                        wr_t = wpool.tile([128, 2, 1024], f32r, tag="wr", name="wr_t")
                        nc.sync.dma_start(
                            wr_t[:],
                            wr_p[ds(kt2 * 256, 256), ds(hg * 1024, 1024)].rearrange(
                                "(k p) n -> p k n", p=128
                            ),
                        )
                        w8_t = w8pool.tile([128, 2, 1024], e4, tag="w8", name="w8_t")
                        nc.gpsimd.dma_start(
                            w8_t[:],
                            w8_p[ds(kt2 * 256, 256), ds(hg * 1024, 1024)].rearrange(
                                "(k p) n -> p k n", p=128
                            ),
                        )
                        dw8_t = w8pool.tile([128, 2, 1024], e4, tag="dw8", name="dw8_t")
                        nc.gpsimd.dma_start(
                            dw8_t[:],
                            dw8_p[ds(kt2 * 256, 256), ds(hg * 1024, 1024)].rearrange(
                                "(k p) n -> p k n", p=128
                            ),
                        )
                        xtr_t = xpool.tile([128, 2, BC], f32r, tag="xtr", name="xtr_t")
                        nc.scalar.dma_start(
                            xtr_t[:],
                            xtr_p[ds(kt2 * 256, 256), :].rearrange(
                                "(k p) b -> p k b", p=128
                            ),
                        )
                        if hg == 0:
                            if kt2 == 0:
                                _load_x8_chunk(0)
                            if kt2 + 1 < KT2:
                                _load_x8_chunk(kt2 + 1)

                        first = kt2 == 0
                        last = kt2 == KT2 - 1
                        for kk in range(2):
                            for m in range(MB):
                                for j in range(2):
                                    nc.tensor.matmul(
                                        pss[m][j][:],
                                        xtr_t[:, kk, ds(m * 128, 128)],
                                        wr_t[:, kk, ds(j * 512, 512)],
                                        start=(first and kk == 0),
                                        stop=False,
                                    )
                        for m in range(MB):
                            for j in range(2):
                                nc.tensor.matmul(
                                    pss[m][j][:],
                                    dx8[:, ds(kt2 * 2, 2), ds(m * 128, 128)],
                                    w8_t[:, :, ds(j * 512, 512)],
                                    start=False,
                                    stop=False,
                                    perf_mode=DR,
                                )
                        for m in range(MB):
                            for j in range(2):
                                nc.tensor.matmul(
                                    pss[m][j][:],
                                    x8[:, ds(kt2 * 2, 2), ds(m * 128, 128)],
                                    dw8_t[:, :, ds(j * 512, 512)],
                                    start=False,
                                    stop=last,
                                    perf_mode=DR,
                                )
                    # free all 8 PSUM banks first (bias adds), then do the
                    # candidate extraction on the SBUF staging tiles so the
                    # next h-group's matmuls aren't gated on DVE extract work
                    sts = {}
                    for m in range(MB):
                        for j in range(2):
                            st = evac.tile([128, 512], f32, tag="est", name="est")
                            if (m * 2 + j) % 2 == 0:
                                nc.vector.tensor_add(st[:], pss[m][j][:], benc_t[:, j])
                            else:
                                nc.scalar.activation(
                                    st[:],
                                    pss[m][j][:],
                                    mybir.ActivationFunctionType.Identity,
                                    bias=0.0,
                                    scale=1.0,
                                )
                                nc.vector.tensor_add(st[:], st[:], benc_t[:, j])
                            nc.gpsimd.dma_start(
                                pre_dram[m, :, ds(hg * 1024 + j * 512, 512)], st[:]
                            )
                            sts[(m, j)] = st
                    for m in range(MB):
                        for j in range(2):
                            # top-16 candidate extraction for this 512 chunk
                            st = sts[(m, j)]
                            c = hg * 2 + j
                            cs = cscr.tile([128, 512], f32, tag="cs", name="cs")
                            nc.vector.max(cands[m][:, c, ds(0, 8)], st[:])
                            nc.vector.match_replace(
                                cs[:], cands[m][:, c, ds(0, 8)], st[:], 0.0
                            )
                            nc.vector.max(cands[m][:, c, ds(8, 8)], cs[:])
                    # incremental candidate compaction: keep the running
                    # top-64 so the final post-encode merge scans only 128
                    # values (chunks 24..31) instead of all 512
                    if hg == 7:
                        for m in range(MB):
                            _compact_cands(m, 0, 16)
                    elif hg == 13:
                        for m in range(MB):
                            _compact_cands(m, 12, 28)

            # ----- phases 2+3: threshold -> hidden^T (xbar DMA) -> decode -----
            # one container so the decode weight stream can prefetch during
            # the threshold phase; hidden^T lives in four [128, KH, 128]
            # per-m tiles whose per-(e, m) transpose destinations are
            # contiguous (required for correct xbar DMA transposes)
            with tc.tile_pool(name="htres", bufs=1) as htres:
                # per-(m, e) tiles so each xbar transpose writes one whole
                # tile — decode's kh-chunk deps are then trivially precise
                # and its first matmuls only wait for the e=0 transposes
                NE = H // 2048  # 8
                hts = [
                    [
                        htres.tile([128, 16, 128], ddt, name=f"ht_{m}_{e}")
                        for e in range(NE)
                    ]
                    for m in range(MB)
                ]

                with (
                    tc.tile_pool(name="prel", bufs=2) as prel,
                    tc.tile_pool(name="hidp", bufs=3) as hidp,
                    tc.tile_pool(name="wdp", bufs=8) as wdp,
                    tc.tile_pool(name="dpsum", bufs=8, space="PSUM") as dps,
                    tc.tile_pool(name="devac", bufs=6) as devac,
                    tc.tile_pool(name="dbias", bufs=2) as dbias,
                ):
                    E8 = 2048
                    NG = D // 1024  # 8
                    # 8-round merge over the surviving candidates (compacted
                    # top-64 in chunks 24..27 + fresh chunks 28..31) -> 64th
                    for m in range(MB):
                        for r in range(8):
                            nc.vector.max(m8s[m][:], cands[m][:, ds(24, 8), :])
                            if r < 7:
                                nc.vector.match_replace(
                                    cands[m][:, ds(24, 8), :],
                                    m8s[m][:],
                                    cands[m][:, ds(24, 8), :],
                                    0.0,
                                )

                    # prefetch the first decode weight tiles + bias so the
                    # tensor engine can roll straight into decode
                    PREF = 6
                    bdec0 = dbias.tile([128, 2, 512], f32, tag="bdec", name="bdec_t")
                    nc.scalar.dma_start(
                        bdec0[:],
                        bdec_p[ds(0, 1024)]
                        .rearrange("(j n) -> j n", n=512)
                        .partition_broadcast(128),
                    )
                    wd_pref = []
                    for kh in range(PREF):
                        wd_t = wdp.tile([128, 1024], ddt, tag="wd", name="wd_t")
                        eng = nc.sync if kh % 2 == 0 else nc.gpsimd
                        eng.dma_start(
                            wd_t[:], wd_p[ds(kh * 128, 128), ds(0, 1024)]
                        )
                        wd_pref.append(wd_t)

                    # threshold-apply, then xbar-transpose each [128, 2048]
                    # hidden tile into the decode layout (no PE involvement)
                    for e in range(NE):
                        for m in range(MB):
                            t_ap = m8s[m][:, ds(7, 1)]
                            pt = prel.tile([128, E8], f32, tag="pt", name="pt")
                            nc.scalar.dma_start(
                                pt[:], pre_dram[m, :, ds(e * E8, E8)]
                            )
                            hd = hidp.tile([128, E8], ddt, tag="hd", name="hd")
                            nc.vector.scalar_tensor_tensor(
                                hd[:], pt[:], t_ap, pt[:], is_ge, mult
                            )
                            nc.scalar.dma_start_transpose(
                                hts[m][e][:], hd[:]
                            )

                    # ---------------- phase 3: decode ----------------
                    for ng in range(NG):
                        pss = [
                            [
                                dps.tile(
                                    [128, 512], f32, name=f"dps_{m}_{j}", tag="dps"
                                )
                                for j in range(2)
                            ]
                            for m in range(MB)
                        ]
                        if ng == 0:
                            bdec_t = bdec0
                        else:
                            bdec_t = dbias.tile(
                                [128, 2, 512], f32, tag="bdec", name="bdec_t"
                            )
                            nc.scalar.dma_start(
                                bdec_t[:],
                                bdec_p[ds(ng * 1024, 1024)]
                                .rearrange("(j n) -> j n", n=512)
                                .partition_broadcast(128),
                            )
                        for kh in range(KH):
                            if ng == 0 and kh < PREF:
                                wd_t = wd_pref[kh]
                            else:
                                wd_t = wdp.tile(
                                    [128, 1024], ddt, tag="wd", name="wd_t"
                                )
                                eng = nc.sync if kh % 2 == 0 else nc.gpsimd
                                eng.dma_start(
                                    wd_t[:],
                                    wd_p[ds(kh * 128, 128), ds(ng * 1024, 1024)],
                                )
                            first = kh == 0
                            last = kh == KH - 1
                            for m in range(MB):
                                for j in range(2):
                                    nc.tensor.matmul(
                                        pss[m][j][:],
                                        hts[m][kh // 16][:, kh % 16, :],
                                        wd_t[:, ds(j * 512, 512)],
                                        start=first,
                                        stop=last,
                                    )
                        for m in range(MB):
                            for j in range(2):
                                st = devac.tile([128, 512], f32, tag="dst", name="dst")
                                # split psum-freeing adds across DVE and ACT so
                                # the next ng group's matmuls unblock sooner
                                if (m * 2 + j) % 2 == 0:
                                    nc.vector.tensor_add(
                                        st[:], pss[m][j][:], bdec_t[:, j]
                                    )
                                else:
                                    nc.scalar.activation(
                                        st[:],
                                        pss[m][j][:],
                                        mybir.ActivationFunctionType.Identity,
                                        bias=0.0,
                                        scale=1.0,
                                    )
                                    nc.vector.tensor_add(st[:], st[:], bdec_t[:, j])
                                nc.scalar.dma_start(
                                    out_p[
                                        ds(m * 128, 128), ds(ng * 1024 + j * 512, 512)
                                    ],
                                    st[:],
                                )

    nc.compile()
    return nc


def prepare_inputs(X, W_enc, W_dec, b_enc, b_dec, n_cores=N_CORES):
    """Host-side dtype splits + per-core sharding. X: [B, D]."""
    B, D = X.shape
    BC = B // n_cores

    Wr = round_fp32r(W_enc)
    wr = (Wr * np.float32(2048.0)).astype(np.float32)  # fp32r * 2^11
    w8 = _e4(W_enc * np.float32(2.0**9))
    dw8 = _e4((W_enc - Wr) * np.float32(2.0**22))
    wd = _bf16(np.asarray(W_dec, np.float32) * np.float32(2.0**-22))
    benc = (np.ascontiguousarray(b_enc, np.float32) * np.float32(2.0**22)).astype(
        np.float32
    )
    bdec = np.ascontiguousarray(b_dec, np.float32).reshape(D)

    in_maps = []
    for c in range(n_cores):
        XT = np.ascontiguousarray(X[c * BC : (c + 1) * BC].T)  # [D, BC]
        XTr = round_fp32r(XT)
        in_maps.append(
            {
                "xtr": (XTr * np.float32(2048.0)).astype(np.float32),
                "x8": _e4(XT),
                "dx8": _e4((XT - XTr) * np.float32(2.0**13)),
                "wr": wr,
                "w8": w8,
                "dw8": dw8,
                "wd": wd,
                "benc": benc,
                "bdec": bdec,
            }
        )
    return in_maps


_NC_CACHE = {}


def _get_nc(**kw):
    key = tuple(sorted(kw.items()))
    if key not in _NC_CACHE:
        _NC_CACHE[key] = build_nc(**kw)
    return _NC_CACHE[key]


def kernel(activation_BMLD, W_enc_MLDH, W_dec_HMLD, b_enc_H, b_dec_MLD, k, **run_kw):
    assert int(k) == _TOPK
    B = activation_BMLD.shape[0]
    X = np.ascontiguousarray(activation_BMLD, np.float32).reshape(B, _D)
    W_enc = np.ascontiguousarray(W_enc_MLDH, np.float32).reshape(_D, _H)
    W_dec = np.ascontiguousarray(W_dec_HMLD, np.float32).reshape(_H, _D)

    nc = _get_nc(BC=B // N_CORES)
    in_maps = prepare_inputs(X, W_enc, W_dec, b_enc_H, b_dec_MLD)
    res = run_bass_kernel_spmd(nc, in_maps, core_ids=list(range(N_CORES)), **run_kw)
    out = np.concatenate([res.results[c]["out"] for c in range(N_CORES)], axis=0)
    if run_kw.get("trace"):
        kernel.last_result = res
    return out.reshape(B, _M, _L, _DM).astype(np.float32)
